# revision 29
# baseline (speedup 1.0000x reference)
"""MetaPathGNN forward on 8 Trainium2 NeuronCores (Bass/Tile).

v4 strategy (self-contained; N=100000, C=256, OUT=128, E=400000, 8 cores):
  - Nodes sharded 12500/core; edges assigned to cores by src owner; host
    sorts each core's edges into a unified slot layout (identical across
    cores -> single SPMD NEFF).
  - LAYER 1 needs NO collective: the full x is replicated to every core as
    an input table; cores gather raw x[dst] rows (GpSimd dma_gather, 512B
    fp16 rows, int16 window-local indices), segment-sum via fp8-selector
    matmuls into PSUM, then apply wl1^T AFTER aggregation (the per-node
    1/deg scale commutes with the matmul).
  - LAYER 2 messages hw2' = h1r@wl0T_s + const1 are computed fused into
    layer-1's per-supertile epilogue and AllGathered in 2 chunks so the
    collective overlaps the tail of layer-1 compute.
  - Epilogues fuse deg-normalize + dense term + bias, relu + LN stats,
    normalize; layer-2 epilogue also fuses the final projection.
"""
import numpy as np
from contextlib import ExitStack

N = 100000
C = 256
OUT = 128
NCORES = 8
NPC = N // NCORES          # 12500 nodes per core
P = 128
TILES = (NPC + P - 1) // P  # 98
NPC_PAD = TILES * P         # 12544
ST_TILES = 4                # layer-1 node-tiles per super-tile
NST = (TILES + ST_TILES - 1) // ST_TILES  # 25
ST2 = 8                     # layer-2 node-tiles per super-tile
NST2 = (TILES + ST2 - 1) // ST2           # 13
WIN1 = 25000                # layer-1 window stride over replicated x table
NWIN1 = 4
NFULL = 100352              # padded full-x rows (784 tiles)
NWIN2 = 4                   # layer-2 windows: one per core pair (padded rows)
LN_EPS = 1e-5
PREP_TAG = 6

_COMPILED = {}


# ---------------------------------------------------------------- host side
def _sigmoid(x):
    return 1.0 / (1.0 + np.exp(-np.float64(x)))


def _win1(d):
    w = d // WIN1
    return w, d - w * WIN1


def _win2(d):
    # table2 = concat over cores of [NPC_PAD, C] (pad rows never referenced)
    c = d // NPC
    r = d - c * NPC
    w = c >> 1
    widx = (c & 1) * NPC_PAD + r
    return w, widx


def _build_layer(src, dst, nwin, win_fn, st_tiles):
    """Unified slot layout builder (see baseline docstring)."""
    nst = (TILES + st_tiles - 1) // st_tiles
    per_core = []
    diag = np.zeros((NCORES, 128, TILES * 128), dtype=np.uint8)
    for c in range(NCORES):
        lo = c * NPC
        m = (src >= lo) & (src < lo + NPC)
        selfm = m & (src == dst)
        si = (src[selfm] - lo).astype(np.int64)
        mult = np.bincount(si, minlength=NPC_PAD)
        pp = np.arange(NPC_PAD)
        diag[c, pp & 127, (pp >> 7) * 128 + (pp & 127)] = mult
        m = m & (src != dst)
        s = (src[m] - lo).astype(np.int64)
        d = dst[m].astype(np.int64)
        w, widx = win_fn(d)
        t = s >> 7
        sti = t // st_tiles
        order = np.lexsort((s, t, w, sti))
        per_core.append((s[order], widx[order], w[order], t[order], sti[order]))

    cnt = np.zeros((NCORES, nst, nwin, TILES), dtype=np.int64)
    for c in range(NCORES):
        s, widx, w, t, sti = per_core[c]
        np.add.at(cnt[c], (sti, w, t), 1)
    ucnt = cnt.max(axis=0)

    structure = []
    total_slots = 0
    total_mms = 0
    slot_tile_all = []
    seg_start = {}
    for sti in range(nst):
        st_runs = []
        for wi in range(nwin):
            segs = [(ti, int(ucnt[sti, wi, ti]))
                    for ti in range(sti * st_tiles, min((sti + 1) * st_tiles, TILES))
                    if ucnt[sti, wi, ti] > 0]
            slots = []
            tiles_in_blk = set()
            f = 0
            for (ti, n) in segs:
                if f > 0 and len(tiles_in_blk) >= 2 and ti not in tiles_in_blk:
                    slots.extend([-1] * (128 - f))
                    f = 0
                    tiles_in_blk = set()
                seg_start[(sti, wi, ti)] = total_slots + len(slots)
                rem = n
                while rem > 0:
                    take = min(128 - f, rem)
                    slots.extend([ti] * take)
                    f += take
                    rem -= take
                    tiles_in_blk.add(ti)
                    if f == 128:
                        f = 0
                        tiles_in_blk = set()
            if f > 0:
                slots.extend([-1] * (128 - f))
            n_slots = len(slots)
            nblk = n_slots // 128
            mm_list = []
            for b in range(nblk):
                blk = slots[b * 128:(b + 1) * 128]
                touched = []
                for q in blk:
                    if q >= 0 and q not in touched:
                        touched.append(q)
                for ti in touched:
                    mm_list.append((b, ti))
            st_runs.append({"w": wi, "n_slots": n_slots, "mm_list": mm_list,
                            "slot_base": total_slots, "mm_base": total_mms})
            slot_tile_all.extend(slots)
            total_slots += n_slots
            total_mms += len(mm_list)
        structure.append(st_runs)

    slot_tile_all = np.asarray(slot_tile_all, dtype=np.int64)

    idx = np.zeros((NCORES, total_slots), dtype=np.int16)
    selcol = np.full((NCORES, total_slots), -1, dtype=np.int64)
    for c in range(NCORES):
        s, widx, w, t, sti = per_core[c]
        key = (sti * nwin + w) * TILES + t
        change = np.empty(len(key), dtype=bool)
        if len(key):
            change[0] = True
            change[1:] = key[1:] != key[:-1]
        grp_start_idx = np.flatnonzero(change)
        grp_of_edge = np.cumsum(change) - 1
        offset_in_grp = np.arange(len(key)) - grp_start_idx[grp_of_edge]
        base = np.array([seg_start[(int(sti[i]), int(w[i]), int(t[i]))]
                         for i in grp_start_idx], dtype=np.int64)
        slot_pos = base[grp_of_edge] + offset_in_grp
        idx[c, slot_pos] = widx.astype(np.int16)
        selcol[c, slot_pos] = s & 127

    sel = np.zeros((NCORES, 128, total_mms * 128), dtype=np.uint8)
    mm_i_global = 0
    for sti in range(nst):
        for run in structure[sti]:
            sb = run["slot_base"]
            for (b, ti) in run["mm_list"]:
                sl0 = sb + b * 128
                tile_match = slot_tile_all[sl0:sl0 + 128] == ti
                for c in range(NCORES):
                    cols = selcol[c, sl0:sl0 + 128]
                    jj = np.flatnonzero(tile_match & (cols >= 0))
                    sel[c, jj, mm_i_global * 128 + cols[jj]] = 1
                mm_i_global += 1
    assert mm_i_global == total_mms

    assert total_slots % 128 == 0
    idx_w = np.zeros((NCORES, 16, total_slots // 16), dtype=np.int16)
    ar = np.arange(total_slots)
    idx_w[:, ar % 16, ar // 16] = idx
    idx_w = np.tile(idx_w, (1, 8, 1))

    return {"structure": structure, "total_slots": total_slots, "total_mms": total_mms,
            "idx": idx_w, "sel": sel, "diag": diag}


def _meta_layout(lay1, lay2):
    S1, S2 = lay1["total_slots"], lay2["total_slots"]
    M1, M2 = lay1["total_mms"], lay2["total_mms"]
    off = {}
    k = 0

    def seg(name, ncols, align=128):
        nonlocal k
        k = -(-k // align) * align
        off[name] = k
        k += ncols

    seg("xT", 2 * NPC_PAD)
    seg("xrows", TILES * C)
    seg("idx1", S1 // 16)
    seg("idx2", S2 // 16)
    seg("sel1", M1 * 64)
    seg("sel2", M2 * 64)
    seg("diag1", TILES * 64)
    seg("diag2", TILES * 64)
    seg("invdeg1", 2 * TILES)
    seg("invdeg2", 2 * TILES)
    seg("wl1T", 512)
    seg("rhsB1", 512)
    seg("rhsA2", 1024)
    seg("rhsY2", 512)
    seg("rhsF", 256)
    seg("brow1", 256)
    seg("crow2", 512)
    seg("browF", 128)
    K = -(-k // 128) * 128
    return off, K


def _kblock(w):
    """[256, F] fp arr -> [128, 2*F] int16 view (k-block-major)."""
    f16 = np.float16
    a = np.ascontiguousarray(w, dtype=np.float32).astype(f16)
    F = a.shape[1]
    return np.ascontiguousarray(
        a.reshape(2, 128, F).transpose(1, 0, 2).reshape(128, 2 * F)).view(np.int16)


def _prep(inputs):
    import ml_dtypes
    fp8 = ml_dtypes.float8_e4m3
    f16 = np.float16
    x = np.asarray(inputs["x"], np.float32)
    ei1 = np.asarray(inputs["edge_index_r1"])
    ei0 = np.asarray(inputs["edge_index_r0"])

    g1 = np.float32(_sigmoid(inputs["gate1"]))
    g0 = np.float32(_sigmoid(inputs["gate0"]))
    lns1 = np.asarray(inputs["lns1"], np.float32); lnb1 = np.asarray(inputs["lnb1"], np.float32)
    lns0 = np.asarray(inputs["lns0"], np.float32); lnb0 = np.asarray(inputs["lnb0"], np.float32)

    wl1 = np.asarray(inputs["wl1"], np.float32); bl1 = np.asarray(inputs["bl1"], np.float32)
    w01 = np.asarray(inputs["w01"], np.float32); b01 = np.asarray(inputs["b01"], np.float32)
    w11 = np.asarray(inputs["w11"], np.float32); b11 = np.asarray(inputs["b11"], np.float32)
    wl0 = np.asarray(inputs["wl0"], np.float32); bl0 = np.asarray(inputs["bl0"], np.float32)
    w00 = np.asarray(inputs["w00"], np.float32); b00 = np.asarray(inputs["b00"], np.float32)
    w10 = np.asarray(inputs["w10"], np.float32); b10 = np.asarray(inputs["b10"], np.float32)
    Wout = np.asarray(inputs["Wout"], np.float32); bout = np.asarray(inputs["bout"], np.float32)

    # Layer 1 (r1 edges): msg = (invdeg1 * sum x[d]) @ wl1T; z = x@W_eff + bias1
    W_eff1 = ((1 - g1) * w01 + g1 * w11).T                 # [256,256]
    bias1 = bl1 + (1 - g1) * b01 + g1 * b11
    # Layer 2 (r0 edges): table rows hw2' = h1r@wl0T_s + const1 (+z2 dense)
    wl0T_s = (lns1[:, None] * wl0.T)
    const1 = lnb1 @ wl0.T
    w00T_s = (1 - g0) * (lns1[:, None] * w00.T)
    rhsA2 = np.concatenate([wl0T_s, w00T_s], axis=1)       # [256,512]
    w10T_s = g0 * w10.T
    bias2 = bl0 + (1 - g0) * (b00 + lnb1 @ w00.T) + g0 * b10
    crow2 = np.concatenate([const1, bias2])                # [512]
    WoutT_s = lns0[:, None] * Wout.T                       # [256,128]
    bout_s = bout + lnb0 @ Wout.T

    inv1 = 1.0 / np.clip(np.bincount(ei1[1], minlength=N), 1.0, None).astype(np.float32)
    inv0 = 1.0 / np.clip(np.bincount(ei0[1], minlength=N), 1.0, None).astype(np.float32)

    lay1 = _build_layer(ei1[0].astype(np.int64), ei1[1].astype(np.int64), NWIN1, _win1, ST_TILES)
    lay2 = _build_layer(ei0[0].astype(np.int64), ei0[1].astype(np.int64), NWIN2, _win2, ST2)

    off, K = _meta_layout(lay1, lay2)

    xfull = np.zeros((NFULL, C), f16)
    xfull[:N] = x.astype(f16)

    def put(blob, name, arr_i16):
        o = off[name]
        blob[:arr_i16.shape[0], o:o + arr_i16.shape[1]] = arr_i16

    in_maps = []
    for c in range(NCORES):
        lo = c * NPC
        blob = np.zeros((128, K), np.int16)
        xs = np.zeros((NPC_PAD, C), np.float32)
        xs[:NPC] = x[lo:lo + NPC]
        xs16 = xs.astype(f16)
        put(blob, "xT", _kblock(np.ascontiguousarray(xs16.T)))
        put(blob, "xrows", np.ascontiguousarray(
            xs16.reshape(TILES, P, C).transpose(1, 0, 2).reshape(P, TILES * C)).view(np.int16))
        put(blob, "idx1", lay1["idx"][c])
        put(blob, "idx2", lay2["idx"][c])
        put(blob, "sel1", lay1["sel"][c].astype(np.float32).astype(fp8).view(np.int16))
        put(blob, "sel2", lay2["sel"][c].astype(np.float32).astype(fp8).view(np.int16))
        put(blob, "diag1", lay1["diag"][c].astype(np.float32).astype(fp8).view(np.int16))
        put(blob, "diag2", lay2["diag"][c].astype(np.float32).astype(fp8).view(np.int16))
        inv1c = np.zeros(NPC_PAD, np.float32); inv1c[:NPC] = inv1[lo:lo + NPC]
        inv0c = np.zeros(NPC_PAD, np.float32); inv0c[:NPC] = inv0[lo:lo + NPC]
        put(blob, "invdeg1",
            np.ascontiguousarray(inv1c.reshape(TILES, P).T).view(np.int16))
        put(blob, "invdeg2",
            np.ascontiguousarray(inv0c.reshape(TILES, P).T).view(np.int16))
        put(blob, "wl1T", _kblock(wl1.T))
        put(blob, "rhsB1", _kblock(W_eff1))
        put(blob, "rhsA2", _kblock(rhsA2))
        put(blob, "rhsY2", _kblock(w10T_s))
        put(blob, "rhsF", _kblock(WoutT_s))
        blob[0:1, off["brow1"]:off["brow1"] + 256] = bias1[None, :].astype(f16).view(np.int16)
        blob[0:1, off["crow2"]:off["crow2"] + 512] = crow2[None, :].astype(f16).view(np.int16)
        blob[0:1, off["browF"]:off["browF"] + 128] = bout_s[None, :].astype(f16).view(np.int16)
        in_maps.append(dict(meta=blob, xfull=xfull))
    return in_maps, lay1, lay2, off, K


# ---------------------------------------------------------------- device side
def _build_nc(lay1, lay2, off, K):
    import concourse.bass as bass
    import concourse.tile as tile
    from concourse import bacc, mybir
    from concourse.masks import make_identity

    f32, f16 = mybir.dt.float32, mybir.dt.float16
    f8, i16 = mybir.dt.float8e4, mybir.dt.int16
    AF = mybir.ActivationFunctionType
    OP = mybir.AluOpType

    nc = bacc.Bacc("TRN2", target_bir_lowering=False, debug=False, num_devices=NCORES)

    S1, S2 = lay1["total_slots"], lay2["total_slots"]
    M1, M2 = lay1["total_mms"], lay2["total_mms"]

    meta = nc.dram_tensor("meta", [P, K], i16, kind="ExternalInput").ap()
    xfull = nc.dram_tensor("xfull", [NFULL, C], f16, kind="ExternalInput").ap()
    out_dram = nc.dram_tensor("out", [NPC_PAD, OUT], f32, kind="ExternalOutput").ap()

    def fslice(name, n):
        o = off[name]
        return meta[:, o:o + n]

    xT_view = fslice("xT", 2 * NPC_PAD).bitcast(f16).rearrange(
        "k (b n) -> k b n", b=2)                        # [128, 2, NPC_PAD]
    xrows_view = fslice("xrows", TILES * C).bitcast(f16)  # [128, TILES*256]
    sel1_view = fslice("sel1", M1 * 64).bitcast(f8)
    sel2_view = fslice("sel2", M2 * 64).bitcast(f8)
    diag1_view = fslice("diag1", TILES * 64).bitcast(f8)
    diag2_view = fslice("diag2", TILES * 64).bitcast(f8)

    ag2 = nc.dram_tensor("ag2", [NPC_PAD, C], f16)
    table2 = nc.dram_tensor("table2", [NCORES * NPC_PAD, C], f16, addr_space="Shared")

    max_blk_per_st = 0
    max_mm_per_st = 0
    for lay in (lay1, lay2):
        for st_runs in lay["structure"]:
            max_blk_per_st = max(max_blk_per_st,
                                 sum(r["n_slots"] for r in st_runs) // 128)
            max_mm_per_st = max(max_mm_per_st,
                                sum(len(r["mm_list"]) for r in st_runs))

    with tile.TileContext(nc) as tc, ExitStack() as ctx:
        sb = ctx.enter_context(tc.tile_pool(name="sb", bufs=1))
        lhs_pool = ctx.enter_context(tc.tile_pool(name="lhs", bufs=4))
        small = ctx.enter_context(tc.tile_pool(name="small", bufs=2))
        gpool = ctx.enter_context(tc.tile_pool(name="gst", bufs=2))
        spool = ctx.enter_context(tc.tile_pool(name="sel", bufs=2))
        epi = ctx.enter_context(tc.tile_pool(name="epi", bufs=4))

        z_sb = sb.tile([P, TILES * C], f16)
        h_sb = sb.tile([P, TILES * C], f16)
        s1_all = sb.tile([P, TILES], f32)
        s2_all = sb.tile([P, TILES], f32)
        mu_all = sb.tile([P, TILES], f32)
        rstd_all = sb.tile([P, TILES], f32)
        invdeg1_sb = sb.tile([P, TILES], f32)
        invdeg2_sb = sb.tile([P, TILES], f32)
        ident16 = sb.tile([P, P], f16)
        make_identity(nc, ident16[:])
        ones_col = sb.tile([1, P], f16)
        nc.vector.memset(ones_col[:], 1.0)
        nc.sync.dma_start(invdeg1_sb[:], fslice("invdeg1", 2 * TILES).bitcast(f32))
        nc.sync.dma_start(invdeg2_sb[:], fslice("invdeg2", 2 * TILES).bitcast(f32))

        def wload(name, ncols, shape):
            tl = sb.tile(shape, f16, tag=f"w_{name}")
            nc.sync.dma_start(tl[:], fslice(name, ncols).bitcast(f16).rearrange(
                "k (b n) -> k b n", b=2))
            return tl

        wl1T_sb = wload("wl1T", 512, [P, 2, 256])
        rhsB1_sb = wload("rhsB1", 512, [P, 2, 256])
        rhsA2_sb = wload("rhsA2", 1024, [P, 2, 512])
        rhsY2_sb = wload("rhsY2", 512, [P, 2, 256])
        rhsF_sb = wload("rhsF", 256, [P, 2, OUT])
        brow1_sb = sb.tile([1, 256], f16)
        nc.sync.dma_start(brow1_sb[:], meta[0:1, off["brow1"]:off["brow1"] + 256].bitcast(f16))
        crow2_sb = sb.tile([1, 512], f16)
        nc.sync.dma_start(crow2_sb[:], meta[0:1, off["crow2"]:off["crow2"] + 512].bitcast(f16))
        browF_sb = sb.tile([1, OUT], f16)
        nc.sync.dma_start(browF_sb[:], meta[0:1, off["browF"]:off["browF"] + 128].bitcast(f16))

        idx1_sb = sb.tile([P, S1 // 16], i16)
        nc.sync.dma_start(idx1_sb[:], fslice("idx1", S1 // 16))
        idx2_sb = sb.tile([P, S2 // 16], i16)
        nc.sync.dma_start(idx2_sb[:], fslice("idx2", S2 // 16))

        def gathers_for_st(lay, sti, idx_sb, g_sb, tbl_fn, wlo=0, whi=99):
            """Issue dma_gathers for runs of supertile sti with wlo<=w<whi;
            returns mm seq with pass-local block offsets."""
            runs = [r for r in lay["structure"][sti] if wlo <= r["w"] < whi]
            blk_off = 0
            for run in runs:
                ns = run["n_slots"]
                if ns == 0:
                    continue
                in_ap = tbl_fn(run["w"])
                sb0 = run["slot_base"]
                nb = ns // 128
                nc.gpsimd.dma_gather(
                    out_ap=g_sb[:, blk_off * C:(blk_off + nb) * C].rearrange(
                        "p (b c) -> p b c", c=C),
                    in_ap=in_ap,
                    idxs_ap=idx_sb[:, sb0 // 16:(sb0 + ns) // 16],
                    num_idxs=ns, num_idxs_reg=ns, elem_size=C,
                )
                blk_off += nb
            mm_seq = []
            blk_off = 0
            for run in runs:
                for (b, ti) in run["mm_list"]:
                    mm_seq.append((blk_off + b, ti))
                blk_off += run["n_slots"] // 128
            return mm_seq

        def hT_of(src_slice, pool, trp_pool):
            hT = pool.tile([P, 2, P], f16, tag="hT")
            for k in range(2):
                tp = trp_pool.tile([P, P], f16, space="PSUM", tag="trp")
                nc.tensor.transpose(tp[:], src_slice[:, k * P:(k + 1) * P], ident16[:])
                nc.vector.tensor_copy(hT[:, k, :], tp[:])
            return hT

        def st_stats_norm(sti, st_tiles=ST_TILES):
            t0 = sti * st_tiles
            nt = min(st_tiles, TILES - t0)
            sl = slice(t0, t0 + nt)
            nc.vector.tensor_scalar(out=mu_all[:, sl], in0=s1_all[:, sl],
                                    scalar1=1.0 / C, scalar2=None, op0=OP.mult)
            var = small.tile([P, ST2], f32, tag="var")
            nc.vector.tensor_tensor(out=var[:, :nt], in0=mu_all[:, sl], in1=mu_all[:, sl], op=OP.mult)
            nc.vector.scalar_tensor_tensor(out=var[:, :nt], in0=s2_all[:, sl], scalar=1.0 / C,
                                           in1=var[:, :nt], op0=OP.mult, op1=OP.subtract)
            nc.vector.tensor_scalar(out=var[:, :nt], in0=var[:, :nt], scalar1=float(LN_EPS),
                                    scalar2=None, op0=OP.add)
            std = small.tile([P, ST2], f32, tag="std")
            nc.scalar.activation(std[:, :nt], var[:, :nt], AF.Sqrt)
            nc.vector.reciprocal(rstd_all[:, sl], std[:, :nt])
            for t in range(t0, t0 + nt):
                nc.vector.tensor_scalar(
                    out=h_sb[:, t * C:(t + 1) * C], in0=z_sb[:, t * C:(t + 1) * C],
                    scalar1=mu_all[:, t:t + 1], scalar2=rstd_all[:, t:t + 1],
                    op0=OP.subtract, op1=OP.mult)

        # ================= LAYER 1 (no collective) + fused layer-2 dense =====
        # PSUM budget (bank-granular): cps 2 + maccp 2 + trp 2 + a2p 2 = 8.
        with tc.tile_pool(name="cps", bufs=2, space="PSUM") as cps, \
             tc.tile_pool(name="trp", bufs=2, space="PSUM") as trp, \
             tc.tile_pool(name="maccp", bufs=2, space="PSUM") as maccp, \
             tc.tile_pool(name="a2p", bufs=2, space="PSUM") as a2p, \
             tc.tile_pool(name="abe", bufs=4) as abe:
            for sti in range(NST):
                st_t0 = sti * ST_TILES
                st_ntiles = min(ST_TILES, TILES - st_t0)
                st_nmm = sum(len(r["mm_list"]) for r in lay1["structure"][sti])

                g_sb = gpool.tile([P, max_blk_per_st * C], f16, tag="g")
                sel_sb = spool.tile([P, max_mm_per_st * P], f8, tag="s")
                mm_b0 = lay1["structure"][sti][0]["mm_base"]
                if st_nmm:
                    nc.sync.dma_start(sel_sb[:, 0:st_nmm * P],
                                      sel1_view[:, mm_b0 * P:(mm_b0 + st_nmm) * P])
                mm_seq = gathers_for_st(
                    lay1, sti, idx1_sb, g_sb,
                    lambda w: xfull[w * WIN1: w * WIN1 + min(32768, NFULL - w * WIN1), :])

                accs = [cps.tile([P, 512], f32, space="PSUM", tag="agg", name=f"agg{_i}")
                        for _i in range((st_ntiles + 1) // 2)]
                last = {}
                for i, (b, ti) in enumerate(mm_seq):
                    last[ti] = i
                # self-edges: diag x own x rows
                for tl in range(st_ntiles):
                    t = st_t0 + tl
                    xr = epi.tile([P, C], f16, tag="xr")
                    nc.sync.dma_start(xr[:], xrows_view[:, t * C:(t + 1) * C])
                    dg = epi.tile([P, P], f8, tag="dg")
                    nc.sync.dma_start(dg[:], diag1_view[:, t * P:(t + 1) * P])
                    reg = accs[tl // 2][:, (tl % 2) * 256:(tl % 2) * 256 + 256]
                    nc.tensor.matmul(reg, lhsT=dg[:], rhs=xr[:],
                                     start=(tl % 2 == 0), stop=(last.get(t) is None),
                                     skip_group_check=True)
                for i, (b, ti) in enumerate(mm_seq):
                    tl = ti - st_t0
                    reg = accs[tl // 2][:, (tl % 2) * 256:(tl % 2) * 256 + 256]
                    nc.tensor.matmul(
                        reg, lhsT=sel_sb[:, i * P:(i + 1) * P],
                        rhs=g_sb[:, b * C:(b + 1) * C],
                        start=False, stop=(last[ti] == i),
                        skip_group_check=True,
                    )
                # epilogue: macc = (agg*invdeg)@wl1T + x@W_eff + bias1, all in
                # one PSUM group; then relu + LN stats.
                maccs = [maccp.tile([P, 512], f32, space="PSUM", tag="macc", name=f"macc{_i}")
                         for _i in range((st_ntiles + 1) // 2)]
                lhs = []
                for tl in range(st_ntiles):
                    t = st_t0 + tl
                    reg = accs[tl // 2][:, (tl % 2) * 256:(tl % 2) * 256 + 256]
                    aggn = epi.tile([P, C], f16, tag="aggn")
                    nc.vector.tensor_scalar(out=aggn[:], in0=reg,
                                            scalar1=invdeg1_sb[:, t:t + 1],
                                            scalar2=None, op0=OP.mult)
                    aT = hT_of(aggn, abe, trp)
                    lh = lhs_pool.tile([P, 2, P], f16, tag="xTt")
                    nc.sync.dma_start(lh[:], xT_view[:, :, t * P:(t + 1) * P])
                    lhs.append(lh)
                    mreg = maccs[tl // 2][:, (tl % 2) * 256:(tl % 2) * 256 + 256]
                    nc.tensor.matmul(mreg, lhsT=aT[:, 0, :], rhs=wl1T_sb[:, 0, :],
                                     start=(tl % 2 == 0), stop=False, skip_group_check=True)
                    nc.tensor.matmul(mreg, lhsT=aT[:, 1, :], rhs=wl1T_sb[:, 1, :],
                                     start=False, stop=False, skip_group_check=True)
                    nc.tensor.matmul(mreg, lhsT=lh[:, 0, :], rhs=rhsB1_sb[:, 0, :],
                                     start=False, stop=False, skip_group_check=True)
                    nc.tensor.matmul(mreg, lhsT=lh[:, 1, :], rhs=rhsB1_sb[:, 1, :],
                                     start=False, stop=False, skip_group_check=True)
                    nc.tensor.matmul(mreg, lhsT=ones_col[:], rhs=brow1_sb[:],
                                     start=False, stop=True, skip_group_check=True)
                    nc.scalar.activation(z_sb[:, t * C:(t + 1) * C], mreg, AF.Relu,
                                         accum_out=s1_all[:, t:t + 1])
                    sq = epi.tile([P, C], f16, tag="esq")
                    nc.scalar.activation(sq[:], z_sb[:, t * C:(t + 1) * C], AF.Square,
                                         accum_out=s2_all[:, t:t + 1])
                st_stats_norm(sti)
                # fused layer-2 dense: acc2 = [h1r@wl0T_s + const1 | z2]
                for tl in range(st_ntiles):
                    t = st_t0 + tl
                    hT = hT_of(h_sb[:, t * C:(t + 1) * C], abe, trp)
                    lh = lhs[tl]
                    acc2 = a2p.tile([P, 512], f32, space="PSUM", tag="acc2")
                    nc.tensor.matmul(acc2[:], lhsT=hT[:, 0, :], rhs=rhsA2_sb[:, 0, :],
                                     start=True, stop=False)
                    nc.tensor.matmul(acc2[:], lhsT=hT[:, 1, :], rhs=rhsA2_sb[:, 1, :],
                                     start=False, stop=False)
                    nc.tensor.matmul(acc2[:, 256:512], lhsT=lh[:, 0, :], rhs=rhsY2_sb[:, 0, :],
                                     start=False, stop=False, skip_group_check=True)
                    nc.tensor.matmul(acc2[:, 256:512], lhsT=lh[:, 1, :], rhs=rhsY2_sb[:, 1, :],
                                     start=False, stop=False, skip_group_check=True)
                    nc.tensor.matmul(acc2[:], lhsT=ones_col[:], rhs=crow2_sb[:],
                                     start=False, stop=True, skip_group_check=True)
                    hw16 = abe.tile([P, C], f16, tag="hw16")
                    nc.scalar.activation(hw16[:], acc2[:, 0:256], AF.Copy)
                    rows = min(P, NPC - t * P)
                    nc.sync.dma_start(ag2[t * P: t * P + rows, :], hw16[:rows, :])
                    nc.vector.tensor_copy(z_sb[:, t * C:(t + 1) * C], acc2[:, 256:512])
        # Single AllGather: collectives act as all-engine barriers in this
        # framework, so one big transfer beats chunked overlap attempts.
        nc.gpsimd.collective_compute(
            "AllGather", mybir.AluOpType.bypass,
            replica_groups=[list(range(NCORES))],
            ins=[ag2[:].opt()], outs=[table2[:].opt()])

        # ================= LAYER 2 + fused final projection ==================
        def tbl2(w):
            base = w * 2 * NPC_PAD
            return table2[base: base + 2 * NPC_PAD, :]

        # PSUM budget: cps2 4 + ftrp 2 + fap 2 = 8 banks.
        with tc.tile_pool(name="cps2", bufs=4, space="PSUM") as cps2, \
             tc.tile_pool(name="ftrp", bufs=2, space="PSUM") as ftrp, \
             tc.tile_pool(name="fap", bufs=2, space="PSUM") as fap, \
             tc.tile_pool(name="fe", bufs=4) as fe:
            for sti in range(NST2):
                st_t0 = sti * ST2
                st_ntiles = min(ST2, TILES - st_t0)
                st_nmm = sum(len(r["mm_list"]) for r in lay2["structure"][sti])
                g_sb = gpool.tile([P, max_blk_per_st * C], f16, tag="g")
                sel_sb = spool.tile([P, max_mm_per_st * P], f8, tag="s")
                mm_b0 = lay2["structure"][sti][0]["mm_base"]
                if st_nmm:
                    nc.sync.dma_start(sel_sb[:, 0:st_nmm * P],
                                      sel2_view[:, mm_b0 * P:(mm_b0 + st_nmm) * P])
                mm_seq = gathers_for_st(lay2, sti, idx2_sb, g_sb, tbl2)

                accs = [cps2.tile([P, 512], f32, space="PSUM", tag="agg2", name=f"agg2_{_i}")
                        for _i in range((st_ntiles + 1) // 2)]
                last = {}
                for i, (b, ti) in enumerate(mm_seq):
                    last[ti] = i
                for tl in range(st_ntiles):
                    t = st_t0 + tl
                    rows = min(P, NPC - t * P)
                    hwl = fe.tile([P, C], f16, tag="hwl")
                    if rows < P:
                        nc.vector.memset(hwl[:], 0.0)
                    nc.sync.dma_start(hwl[:rows, :], ag2[t * P: t * P + rows, :])
                    dg = fe.tile([P, P], f8, tag="dg")
                    nc.sync.dma_start(dg[:], diag2_view[:, t * P:(t + 1) * P])
                    reg = accs[tl // 2][:, (tl % 2) * 256:(tl % 2) * 256 + 256]
                    nc.tensor.matmul(reg, lhsT=dg[:], rhs=hwl[:],
                                     start=(tl % 2 == 0), stop=(last.get(t) is None),
                                     skip_group_check=True)
                for i, (b, ti) in enumerate(mm_seq):
                    tl = ti - st_t0
                    reg = accs[tl // 2][:, (tl % 2) * 256:(tl % 2) * 256 + 256]
                    nc.tensor.matmul(
                        reg, lhsT=sel_sb[:, i * P:(i + 1) * P],
                        rhs=g_sb[:, b * C:(b + 1) * C],
                        start=False, stop=(last[ti] == i),
                        skip_group_check=True,
                    )
                for tl in range(st_ntiles):
                    t = st_t0 + tl
                    reg = accs[tl // 2][:, (tl % 2) * 256:(tl % 2) * 256 + 256]
                    tmp = fe.tile([P, C], f32, tag="etmp2")
                    nc.vector.scalar_tensor_tensor(
                        out=tmp[:], in0=reg, scalar=invdeg2_sb[:, t:t + 1],
                        in1=z_sb[:, t * C:(t + 1) * C], op0=OP.mult, op1=OP.add)
                    nc.scalar.activation(z_sb[:, t * C:(t + 1) * C], tmp[:], AF.Relu,
                                         accum_out=s1_all[:, t:t + 1])
                    sq = fe.tile([P, C], f16, tag="esq2")
                    nc.scalar.activation(sq[:], z_sb[:, t * C:(t + 1) * C], AF.Square,
                                         accum_out=s2_all[:, t:t + 1])
                st_stats_norm(sti, ST2)
                # fused final projection (paired [P, 2*OUT] PSUM tiles)
                faccs = [fap.tile([P, 2 * OUT], f32, space="PSUM", tag="facc",
                                  name=f"facc{_i}")
                         for _i in range((st_ntiles + 1) // 2)]
                for tl in range(st_ntiles):
                    t = st_t0 + tl
                    hT = hT_of(h_sb[:, t * C:(t + 1) * C], fe, ftrp)
                    freg = faccs[tl // 2][:, (tl % 2) * OUT:(tl % 2) * OUT + OUT]
                    nc.tensor.matmul(freg, lhsT=hT[:, 0, :], rhs=rhsF_sb[:, 0, :],
                                     start=(tl % 2 == 0), stop=False, skip_group_check=True)
                    nc.tensor.matmul(freg, lhsT=hT[:, 1, :], rhs=rhsF_sb[:, 1, :],
                                     start=False, stop=False, skip_group_check=True)
                    nc.tensor.matmul(freg, lhsT=ones_col[:], rhs=browF_sb[:],
                                     start=False, stop=True, skip_group_check=True)
                    o_sb = fe.tile([P, OUT], f32, tag="fo")
                    nc.vector.tensor_copy(o_sb[:], freg)
                    nc.sync.dma_start(out_dram[t * P:(t + 1) * P, :], o_sb[:])

    nc.compile()
    return nc


# ---------------------------------------------------------------- entry point
def kernel(**inputs):
    from concourse.bass_utils import run_bass_kernel_spmd

    in_maps, lay1, lay2, off, K = _prep(inputs)
    key = "nc"
    if key not in _COMPILED:
        _COMPILED[key] = _build_nc(lay1, lay2, off, K)
    nc = _COMPILED[key]
    res = run_bass_kernel_spmd(nc, in_maps, core_ids=list(range(NCORES)))
    _COMPILED["last_res"] = res
    out = np.concatenate([res.results[c]["out"][:NPC] for c in range(NCORES)], axis=0)
    return out.astype(np.float32)
